# revision 33
# baseline (speedup 1.0000x reference)
"""Trainium2 Bass kernel for the DGL-JTNN tree-GRU encoder.

Math note: the reference runs a full up+down message-passing schedule, but the
output only reads h[ROOTS], and a root's in-edges are exactly the up-edges of
its two children.  Up-edge messages depend only on deeper up-edge messages, so
the entire down phase is dead code for the output.  We therefore compute only
the bottom-up pass, level by level over the balanced binary trees.

Layout: everything on-chip is feature-major [H=128 partitions, nodes], with
nodes ordered (tree-major, heap-order within the level).  In that order the
two children of parent column j at level d are columns 2j, 2j+1 of level d+1,
so all graph gathers become stride-2 adds / column-repeat broadcast APs.

Sharding: data-parallel over trees, 8 trees per NeuronCore; the small weight
matrices and the embedding table are replicated (as per the sharding hint).
"""

import os
import sys

import numpy as np

for _p in ("/opt/trn_rl_repo",):
    if os.path.isdir(_p) and _p not in sys.path:
        sys.path.insert(0, _p)

B, DEPTH, H, VOCAB = 64, 10, 128, 780
NPT = 2 ** (DEPTH + 1) - 1
NCORES = 8
T = B // NCORES  # trees per core
CHUNK = 512  # elementwise/ACT chunk width
MMN = 512  # max fp32 matmul moving dim
SCHUNK = 512  # small-level (single-chunk) tile width
GATHER_GROUP = int(os.environ.get("DGLJ_GG", "1024"))  # embedding rows per dma_gather

NCOLS = {d: T * (1 << d) for d in range(DEPTH + 1)}
NPAD = {d: max(NCOLS[d], 128) for d in range(DEPTH + 1)}  # gather pad (idx %128)
IDX_OFF = {}
_off = 0
for _d in range(DEPTH, -1, -1):
    IDX_OFF[_d] = _off
    _off += NPAD[_d] // 16
GIDX_COLS = _off

_W_NAMES = ("wz1", "wz2", "wh1", "wh2", "wr", "ur", "wg1", "wg2")
_B_NAMES = ("bz", "bh", "br", "bg")
WPACK_COLS = 9 * H + len(_B_NAMES) + 1  # +1: negated bz for the zc trick

# float32r = native single-pass fp32 matmul mode (1 row/cycle at N>=256 vs 4
# for the exact 2-pass float32 path).  Validated on HW below; flip via env for
# experiments.
MM_F32R = os.environ.get("DGLJ_MM_F32R", "1") == "1"
SPLIT_TAIL = os.environ.get("DGLJ_SPLIT_TAIL", "1") == "1"
ZC_DVE = os.environ.get("DGLJ_ZC_DVE", "1") == "1"
U_POOL = os.environ.get("DGLJ_U_POOL", "1") == "1"

_NC_CACHE = {}


def _cdiv(a, b):
    return (a + b - 1) // b


def _build_nc(reps=1):
    """Build (and finalize) the per-core Bass program.  Same program for all 8
    cores; only the input data differs (SPMD)."""
    from contextlib import ExitStack

    import concourse.bass as bass
    import concourse.mybir as mybir
    import concourse.tile as tile
    from concourse import bacc

    f32 = mybir.dt.float32
    AF = mybir.ActivationFunctionType

    nc = bacc.Bacc("TRN2", target_bir_lowering=False)

    emb_d = nc.dram_tensor("emb", [VOCAB, H], f32, kind="ExternalInput")
    emb16_d = nc.dram_tensor(
        "emb16", [VOCAB, H], mybir.dt.bfloat16, kind="ExternalInput"
    )
    gidx_d = nc.dram_tensor(
        "gidx", [128, GIDX_COLS], mybir.dt.int16, kind="ExternalInput"
    )
    wpack_d = nc.dram_tensor("wpack", [H, WPACK_COLS], f32, kind="ExternalInput")
    out_d = nc.dram_tensor("out", [H, T], f32, kind="ExternalOutput")

    f32r = mybir.dt.float32r

    def mm(out, lhsT, rhs, start, stop):
        if MM_F32R:
            lhsT = lhsT.bitcast(f32r)
            rhs = rhs.bitcast(f32r)
        nc.tensor.matmul(out, lhsT, rhs, start=start, stop=stop)

    def mm16(out, lhsT, rhs, start, stop):
        nc.tensor.matmul(out, lhsT, rhs, start=start, stop=stop)

    def rnd(ap):
        # Output-AP cast so the producing op rounds to the f32r grid (the
        # verifier requires every fp32r-matmul operand to be pre-rounded).
        return ap.bitcast(f32r) if MM_F32R else ap

    with tile.TileContext(nc) as tc, ExitStack() as ctx:
        consts = ctx.enter_context(tc.tile_pool(name="consts", bufs=1))
        xpool = ctx.enter_context(tc.tile_pool(name="xp", bufs=1))
        mpool = ctx.enter_context(tc.tile_pool(name="mp", bufs=1))
        stpool = ctx.enter_context(tc.tile_pool(name="stage", bufs=3))
        ck = ctx.enter_context(tc.tile_pool(name="ck", bufs=2))
        pzp = ctx.enter_context(tc.tile_pool(name="pz", bufs=3, space="PSUM"))
        php = ctx.enter_context(tc.tile_pool(name="ph", bufs=2, space="PSUM"))
        prp = ctx.enter_context(tc.tile_pool(name="pr", bufs=2, space="PSUM"))
        pxp = ctx.enter_context(tc.tile_pool(name="px", bufs=1, space="PSUM"))

        # ---- constants into SBUF (gidx first: gathers depend on it) ----
        gidx = consts.tile([128, GIDX_COLS], mybir.dt.int16, tag="gidx", name="gidx")
        nc.sync.dma_start(out=gidx[:], in_=gidx_d[:])
        wld = consts.tile([H, WPACK_COLS], f32, tag="wld", name="wld")
        nc.sync.dma_start(out=wld[:], in_=wpack_d[:])
        wrnd = consts.tile([H, 8 * H], f32, tag="wrnd", name="wrnd")
        nc.vector.tensor_copy(rnd(wrnd[:]), wld[:, : 8 * H])
        wsb = {n: wrnd[:, i * H : (i + 1) * H] for i, n in enumerate(_W_NAMES)}
        bf16 = mybir.dt.bfloat16
        w16 = consts.tile([H, 4, H], bf16, tag="w16", name="w16")
        nc.vector.tensor_copy(
            w16[:],
            wld[:, : 8 * H].rearrange("p (w c) -> p w c", c=H)[:, 0::2, :],
        )
        wsb16 = {
            "wz1": w16[:, 0, :],
            "wh1": w16[:, 1, :],
            "wr": w16[:, 2, :],
            "wg1": w16[:, 3, :],
        }
        ident16 = consts.tile([H, H], bf16, tag="id16", name="ident16")
        nc.vector.tensor_copy(ident16[:], wld[:, 8 * H : 9 * H])
        ident = wld[:, 8 * H : 9 * H]
        bsb = {n: wld[:, 9 * H + i : 9 * H + i + 1] for i, n in enumerate(_B_NAMES)}
        nbz = wld[:, 9 * H + 4 : 9 * H + 5]

        xt = {}  # level -> feature-major X tile [128, n_d]
        copy_alt = [0]  # alternate psum->sbuf copy engine

        def gather_level(d):
            """Gather level-d node embeddings (bf16) node-major, then PE
            transpose to feature-major [128, n]."""
            n = NCOLS[d]
            npad = NPAD[d]
            X = xpool.tile(
                [128, npad], mybir.dt.bfloat16, tag=f"x{d % 3}", name=f"x{d}"
            )
            xt[d] = X
            icol0 = IDX_OFF[d]
            # smaller first group at the leaf level so the first transposes
            # (and the whole pipeline) start sooner
            starts = list(range(0, npad, GATHER_GROUP))
            if d == DEPTH:
                starts = [0, 512] + [s + 512 for s in starts[1:-1]] + (
                    [npad - GATHER_GROUP + 512] if npad > GATHER_GROUP else []
                )
                starts = sorted(set(s for s in starts if s < npad))
            for si, i0 in enumerate(starts):
                nxt = starts[si + 1] if si + 1 < len(starts) else npad
                cnt = nxt - i0
                valid = cnt
                st = stpool.tile(
                    [128, GATHER_GROUP // 128, 128],
                    mybir.dt.bfloat16,
                    tag="stage",
                    name="st",
                )
                nc.gpsimd.dma_gather(
                    st[:, : _cdiv(cnt, 128), :],
                    emb16_d[:, :],
                    gidx[:, icol0 + i0 // 16 : icol0 + (i0 + cnt) // 16],
                    num_idxs=cnt,
                    num_idxs_reg=valid,
                    elem_size=H,
                )
                for p0 in range(0, valid, 1024):
                    w = min(1024, valid - p0)
                    pxt = pxp.tile([128, 1024], mybir.dt.bfloat16, tag="px", name="pxt")
                    for t0 in range(0, w, 128):
                        tw = min(128, w - t0)
                        ti = (p0 + t0) // 128
                        nc.tensor.transpose(
                            pxt[:, t0 : t0 + tw],
                            st[:tw, ti, :],
                            ident16[:tw, :tw],
                        )
                    dst = X[:, i0 + p0 : i0 + p0 + w]
                    nc.vector.tensor_copy(dst, pxt[:, :w])
                    copy_alt[0] += 1

        def level_small(d, M, RM, Mn, RMn, lo=0, w=None):
            # Latency-optimized single-chunk path: compute (1-z)*s off the
            # critical chain and feed Ur@m as Ur@a + Ur@(z*h~) so the
            # rm -> next-level chain is as short as possible.  [lo, lo+w) is a
            # tree-aligned column subrange: two half-batches pipeline the
            # otherwise serial level chain.
            n = NCOLS[d]
            if w is None:
                w = n
            X = xt[d]
            Xp = xt[d - 1]
            Mv = Mn[:, 2 * lo : 2 * (lo + w)].rearrange("p (n two) -> p n two", two=2)
            RMv = RMn[:, 2 * lo : 2 * (lo + w)].rearrange("p (n two) -> p n two", two=2)
            S = ck.tile([128, SCHUNK], f32, tag="s", name="S", bufs=3)
            nc.vector.tensor_add(rnd(S[:, :w]), Mv[:, :, 0], Mv[:, :, 1])
            zt = pzp.tile([128, SCHUNK], f32, tag="pz", name="zt")
            mm16(zt[:, :w], wsb16["wz1"], X[:, lo : lo + w], start=True, stop=False)
            mm(zt[:, :w], wsb["wz2"], S[:, :w], start=False, stop=True)
            z = ck.tile([128, SCHUNK], f32, tag="z", name="z", bufs=3)
            nc.scalar.activation(z[:, :w], zt[:, :w], AF.Sigmoid, bias=bsb["bz"])
            zc = ck.tile([128, SCHUNK], f32, tag="z", name="zc", bufs=3)
            if ZC_DVE:
                nc.vector.tensor_scalar(
                    out=zc[:, :w], in0=z[:, :w], scalar1=-1.0, scalar2=1.0,
                    op0=mybir.AluOpType.mult, op1=mybir.AluOpType.add,
                )
            else:
                nc.scalar.activation(
                    zc[:, :w], zt[:, :w], AF.Sigmoid, bias=nbz, scale=-1.0
                )
            a = ck.tile([128, SCHUNK], f32, tag="u", name="a", bufs=3)
            nc.vector.tensor_mul(rnd(a[:, :w]), zc[:, :w], S[:, :w])
            last = d == 1
            if not last:
                rt = prp.tile([128, SCHUNK], f32, tag="pr", name="rt")
                mm(rt[:, :w], wsb["ur"], a[:, :w], start=True, stop=False)
                wp = w // 2
                xpb = Xp[:, lo // 2 : lo // 2 + wp].to_broadcast([128, wp, 2])
                mm16(rt[:, :w], wsb16["wr"], xpb, start=False, stop=False)
            htp = php.tile([128, SCHUNK], f32, tag="ph", name="htp")
            mm16(htp[:, :w], wsb16["wh1"], X[:, lo : lo + w], start=True, stop=False)
            mm(htp[:, :w], wsb["wh2"], RMv[:, :, 0], start=False, stop=False)
            mm(htp[:, :w], wsb["wh2"], RMv[:, :, 1], start=False, stop=True)
            ht = ck.tile([128, SCHUNK], f32, tag="h", name="ht", bufs=3)
            nc.scalar.activation(ht[:, :w], htp[:, :w], AF.Tanh, bias=bsb["bh"])
            t2 = ck.tile([128, SCHUNK], f32, tag="u", name="t2", bufs=3)
            nc.vector.tensor_mul(rnd(t2[:, :w]), z[:, :w], ht[:, :w])
            nc.vector.tensor_add(rnd(M[:, lo : lo + w]), a[:, :w], t2[:, :w])
            if not last:
                mm(rt[:, :w], wsb["ur"], t2[:, :w], start=False, stop=True)
                r = ck.tile([128, SCHUNK], f32, tag="h", name="r", bufs=3)
                nc.scalar.activation(r[:, :w], rt[:, :w], AF.Sigmoid, bias=bsb["br"])
                nc.vector.tensor_mul(rnd(RM[:, lo : lo + w]), r[:, :w], M[:, lo : lo + w])

        def level_compute(d, M, RM, Mn, RMn):
            n = NCOLS[d]
            if n <= SCHUNK and d < DEPTH:
                if SPLIT_TAIL and n >= 32:
                    level_small(d, M, RM, Mn, RMn, lo=0, w=n // 2)
                    level_small(d, M, RM, Mn, RMn, lo=n // 2, w=n // 2)
                else:
                    level_small(d, M, RM, Mn, RMn)
                return
            X = xt[d]
            Xp = xt[d - 1]
            for c0 in range(0, n, CHUNK):
                w = min(CHUNK, n - c0)
                cs = slice(c0, c0 + w)
                leaf = d == DEPTH
                halves = [(q0, min(MMN, w - q0)) for q0 in range(0, w, MMN)]
                if not leaf:
                    Mv = Mn[:, 2 * c0 : 2 * c0 + 2 * w].rearrange(
                        "p (n two) -> p n two", two=2
                    )
                    RMv = RMn[:, 2 * c0 : 2 * c0 + 2 * w].rearrange(
                        "p (n two) -> p n two", two=2
                    )
                    S = ck.tile([128, CHUNK], f32, tag="s", name="S", bufs=3)
                    nc.vector.tensor_add(rnd(S[:, :w]), Mv[:, :, 0], Mv[:, :, 1])
                # z = sigmoid(Wz1 @ x + Wz2 @ s + bz)   (feature-major preacts)
                zt = pzp.tile([128, CHUNK], f32, tag="pz", name="zt")
                for q0, qw in halves:
                    qs = slice(q0, q0 + qw)
                    mm16(zt[:, qs], wsb16["wz1"], X[:, c0 + q0 : c0 + q0 + qw],
                         start=True, stop=leaf)
                    if not leaf:
                        mm(zt[:, qs], wsb["wz2"], S[:, qs], start=False, stop=True)
                z = ck.tile([128, CHUNK], f32, tag="z", name="z", bufs=3)
                nc.scalar.activation(z[:, :w], zt[:, :w], AF.Sigmoid, bias=bsb["bz"])
                # h~ = tanh(Wh1 @ x + Wh2 @ arm + bh); arm pairsum folded into PSUM
                htp = php.tile([128, CHUNK], f32, tag="ph", name="htp")
                for q0, qw in halves:
                    qs = slice(q0, q0 + qw)
                    mm16(htp[:, qs], wsb16["wh1"], X[:, c0 + q0 : c0 + q0 + qw],
                         start=True, stop=leaf)
                    if not leaf:
                        mm(htp[:, qs], wsb["wh2"], RMv[:, q0 : q0 + qw, 0],
                           start=False, stop=False)
                        mm(htp[:, qs], wsb["wh2"], RMv[:, q0 : q0 + qw, 1],
                           start=False, stop=True)
                ht = ck.tile([128, CHUNK], f32, tag="h", name="ht", bufs=3)
                nc.scalar.activation(ht[:, :w], htp[:, :w], AF.Tanh, bias=bsb["bh"])
                # m = s + z * (h~ - s)    (leaf: m = z * h~)
                if leaf:
                    nc.vector.tensor_mul(rnd(M[:, cs]), z[:, :w], ht[:, :w])
                else:
                    u = ck.tile([128, CHUNK], f32, tag="u", name="u", bufs=3)
                    nc.vector.tensor_sub(u[:, :w], ht[:, :w], S[:, :w]) if not U_POOL else nc.gpsimd.tensor_sub(u[:, :w], ht[:, :w], S[:, :w])
                    v = ck.tile([128, CHUNK], f32, tag="v", name="v", bufs=3)
                    nc.vector.tensor_mul(v[:, :w], z[:, :w], u[:, :w])
                    nc.vector.tensor_add(rnd(M[:, cs]), S[:, :w], v[:, :w])
                if d == 1:
                    # rm of level 1 feeds nothing the output needs
                    continue
                # r = sigmoid(Wr @ x_parent + Ur @ m + br); parent cols repeat 2x
                rt = prp.tile([128, CHUNK], f32, tag="pr", name="rt")
                for q0, qw in halves:
                    qs = slice(q0, q0 + qw)
                    mm(rt[:, qs], wsb["ur"], M[:, c0 + q0 : c0 + q0 + qw],
                       start=True, stop=False)
                    qp = qw // 2
                    xpb = Xp[:, (c0 + q0) // 2 : (c0 + q0) // 2 + qp].to_broadcast(
                        [128, qp, 2]
                    )
                    mm16(rt[:, qs], wsb16["wr"], xpb, start=False, stop=True)
                r = ck.tile([128, CHUNK], f32, tag="r", name="r", bufs=3)
                nc.scalar.activation(r[:, :w], rt[:, :w], AF.Sigmoid, bias=bsb["br"])
                nc.gpsimd.tensor_mul(rnd(RM[:, cs]), r[:, :w], M[:, cs])

        # offsets of levels <= SMALL_X_MAX inside the combined xsmall tile
        SMALL_X_MAX = 7
        small_off = {}
        _o = 0
        for _d in range(SMALL_X_MAX, -1, -1):
            small_off[_d] = _o
            _o += NPAD[_d]
        SMALL_COLS = _o

        def gather_small():
            xs = xpool.tile(
                [128, SMALL_COLS], mybir.dt.bfloat16, tag="xs", name="xsmall"
            )
            for _d in range(SMALL_X_MAX, -1, -1):
                xt[_d] = xs[:, small_off[_d] : small_off[_d] + NPAD[_d]]
            icol0 = IDX_OFF[SMALL_X_MAX]
            for i0 in range(0, SMALL_COLS, GATHER_GROUP):
                cnt = min(GATHER_GROUP, SMALL_COLS - i0)
                st = stpool.tile(
                    [128, GATHER_GROUP // 128, 128],
                    mybir.dt.bfloat16,
                    tag="stage",
                    name="st",
                )
                nc.gpsimd.dma_gather(
                    st[:, : _cdiv(cnt, 128), :],
                    emb16_d[:, :],
                    gidx[:, icol0 + i0 // 16 : icol0 + (i0 + cnt) // 16],
                    num_idxs=cnt,
                    num_idxs_reg=cnt,
                    elem_size=H,
                )
                for p0 in range(0, cnt, 1024):
                    w = min(1024, cnt - p0)
                    pxt = pxp.tile([128, 1024], mybir.dt.bfloat16, tag="px", name="pxt")
                    for t0 in range(0, w, 128):
                        tw = min(128, w - t0)
                        ti = (p0 + t0) // 128
                        nc.tensor.transpose(
                            pxt[:, t0 : t0 + tw], st[:tw, ti, :], ident16[:tw, :tw]
                        )
                    nc.vector.tensor_copy(xs[:, i0 + p0 : i0 + p0 + w], pxt[:, :w])

        # ---- schedule ----
        for _rep in range(reps):
            gather_level(DEPTH)
            gather_level(DEPTH - 1)
            gather_small()
            Mn = RMn = None
            M1 = None
            for d in range(DEPTH, 0, -1):
                if DEPTH - 1 >= d - 2 >= 8:
                    gather_level(d - 2)
                M = mpool.tile([128, NCOLS[d]], f32, tag=f"m{d % 2}", name=f"M{d}")
                RM = None
                if d > 1:
                    RM = mpool.tile([128, NCOLS[d]], f32, tag=f"rm{d % 2}", name=f"RM{d}")
                level_compute(d, M, RM, Mn, RMn)
                Mn, RMn = M, RM
                if d == 1:
                    M1 = M
            # ---- root readout: relu(Wg1 @ x_root + Wg2 @ (m_c1 + m_c2) + bg)
            M1v = M1[:, : 2 * T].rearrange("p (n two) -> p n two", two=2)
            S1 = ck.tile([128, T], f32, tag="s", name="S1", bufs=3)
            nc.vector.tensor_add(rnd(S1[:]), M1v[:, :, 0], M1v[:, :, 1])
            pg = pzp.tile([128, T], f32, tag="pz", name="pg")
            mm16(pg[:], wsb16["wg1"], xt[0][:, :T], start=True, stop=False)
            mm(pg[:], wsb["wg2"], S1[:], start=False, stop=True)
            outt = ck.tile([128, T], f32, tag="h", name="outt", bufs=3)
            nc.scalar.activation(outt[:], pg[:], AF.Relu, bias=bsb["bg"])
            nc.sync.dma_start(out=out_d[:, :], in_=outt[:])

    nc.finalize()
    return nc


def get_nc(reps=1):
    key = ("nc", reps)
    if key not in _NC_CACHE:
        _NC_CACHE[key] = _build_nc(reps)
    return _NC_CACHE[key]


def make_core_inputs(wid, emb, weights):
    """Per-core input dicts.  `weights` is the dict of raw weight arrays."""
    wid = np.asarray(wid).reshape(B, NPT)
    wmats = {
        "wz1": weights["Wz_w"][:H],
        "wz2": weights["Wz_w"][H:],
        "wh1": weights["Wh_w"][:H],
        "wh2": weights["Wh_w"][H:],
        "wr": weights["Wr_w"],
        "ur": weights["Ur_w"],
        "wg1": weights["Wg_w"][:H],
        "wg2": weights["Wg_w"][H:],
    }
    bvecs = {
        "bz": weights["Wz_b"],
        "bh": weights["Wh_b"],
        "br": weights["Ur_b"],
        "bg": weights["Wg_b"],
    }
    wpack = np.zeros((H, WPACK_COLS), dtype=np.float32)
    for i, n in enumerate(_W_NAMES):
        wpack[:, i * H : (i + 1) * H] = wmats[n].astype(np.float32)
    wpack[:, 8 * H : 9 * H] = np.eye(H, dtype=np.float32)
    for i, n in enumerate(_B_NAMES):
        wpack[:, 9 * H + i] = bvecs[n].astype(np.float32)
    wpack[:, 9 * H + len(_B_NAMES)] = -bvecs["bz"].astype(np.float32)
    import ml_dtypes

    embf = np.ascontiguousarray(np.asarray(emb, dtype=np.float32))
    base = {
        "emb": embf,
        "emb16": np.ascontiguousarray(embf.astype(ml_dtypes.bfloat16)),
        "wpack": wpack,
    }
    in_maps = []
    for c in range(NCORES):
        widc = wid[c * T : (c + 1) * T]
        blocks = []
        for d in range(DEPTH, -1, -1):
            ids = widc[:, (1 << d) - 1 : (1 << (d + 1)) - 1].reshape(-1)
            ids = ids.astype(np.int16)
            pad = NPAD[d] - len(ids)
            if pad:
                ids = np.concatenate([ids, np.zeros(pad, np.int16)])
            blocks.append(ids.reshape(-1, 16).T)
        gi = np.concatenate(blocks, axis=1)  # [16, GIDX_COLS]
        assert gi.shape == (16, GIDX_COLS), gi.shape
        in_maps.append({**base, "gidx": np.ascontiguousarray(np.tile(gi, (8, 1)))})
    return in_maps


def kernel(**inputs):
    from concourse.bass_utils import run_bass_kernel_spmd

    nc = get_nc()
    in_maps = make_core_inputs(inputs["wid"], inputs["emb"], inputs)
    res = run_bass_kernel_spmd(nc, in_maps, core_ids=list(range(NCORES)))
    out = np.concatenate(
        [np.asarray(res.results[c]["out"]).T for c in range(NCORES)], axis=0
    )
    return np.ascontiguousarray(out.astype(np.float32))
